# revision 3
# baseline (speedup 1.0000x reference)
"""Supervised contrastive loss kernel v3 — Trainium2, 8 cores, Bass/Tile.

Same SPMD-uniform cyclic-symmetric scheme as v2 (see kernel_v2.py docstring)
plus:
  - feat_rows shipped bf16 (faster start, 2x DVE ssq rate)
  - diagonal mask generated on device via affine_select (no wmask input)
  - class-sum matrix C cast to fp8 so the positives matmul reads gT directly
    (drops the gTo bf16 copy and its 16 DVE mults)
  - normalize multiplies alternate DVE/GpSimd (two engines share the work)
  - bulk DMAs issued from the SP (sync) HWDGE queue instead of gpsimd
  - positives emitted before the main loop so they fill the post-AllReduce
    PE gap
NOTE: tensor_tensor_reduce + collective_compute in one program deadlocks on
HW (bisected 2026-08-08) — stick to tensor_mul + reduce_sum.
"""

import numpy as np
import ml_dtypes

import concourse.bass as bass
import concourse.bacc as bacc
import concourse.mybir as mybir
from concourse import tile
from concourse.bass_utils import run_bass_kernel_spmd

N, D, NT, NC = 8192, 1024, 32, 8
KT = D // 128
NCH = 10
T = 0.07
EPS = 1e-10
NEG = -1.0e12

USE_FP8 = True
S_FP8 = 16.0

F32 = mybir.dt.float32
BF16 = mybir.dt.bfloat16
FP8 = mybir.dt.float8e4
BF16_NP = ml_dtypes.bfloat16

RUNS = sorted(
    [(i, d) for i in range(2) for d in range(9)], key=lambda r: (r[0] + r[1], r[0])
)


def build_program():
    S = S_FP8 if USE_FP8 else 1.0
    gdt = FP8 if USE_FP8 else BF16

    nc = bacc.Bacc(None, target_bir_lowering=False, debug=False)
    ftloc = nc.dram_tensor("ftloc", [D, NCH * 512], BF16, kind="ExternalInput")
    feat = nc.dram_tensor("feat_rows", [1024, D], BF16, kind="ExternalInput")
    ohr = nc.dram_tensor("oh_rows", [128, 8 * NT], BF16, kind="ExternalInput")
    identr = nc.dram_tensor("ident", [128, 128], F32, kind="ExternalInput")
    onesr = nc.dram_tensor("ones128", [128, 1], BF16, kind="ExternalInput")
    e8r = nc.dram_tensor("e8sel", [8, 8 * 128], BF16, kind="ExternalInput")
    e16r = nc.dram_tensor("e16sel", [16, NCH * 128], BF16, kind="ExternalInput")

    stats_o = nc.dram_tensor("stats_o", [128, 24], F32, kind="ExternalOutput")
    den_c_o = nc.dram_tensor("den_col_o", [16, 512], F32, kind="ExternalOutput")

    AX = mybir.AxisListType.X
    ADD = mybir.AluOpType.add
    AF = mybir.ActivationFunctionType
    DR = mybir.MatmulPerfMode.DoubleRow

    with tile.TileContext(nc) as tc:
        with (
            tc.tile_pool(name="dram", bufs=1, space="DRAM") as dpool,
            tc.tile_pool(name="big", bufs=1) as big,
            tc.tile_pool(name="stage", bufs=3) as stage,
            tc.tile_pool(name="spool", bufs=2) as spool,
            tc.tile_pool(name="epool", bufs=3) as epool,
            tc.tile_pool(name="cpool", bufs=2) as cpool,
            tc.tile_pool(name="scr", bufs=2) as scr,
            tc.tile_pool(name="psim", bufs=5, space="PSUM") as psim,
            tc.tile_pool(name="pcol", bufs=2, space="PSUM") as pcol,
            tc.tile_pool(name="paux", bufs=1, space="PSUM") as paux,
        ):
            scl_d = dpool.tile([8, 128], BF16, tag="scl_d")
            scl_all = dpool.tile([16, 512], BF16, tag="scl_all", addr_space="Shared")
            cpart_d = dpool.tile([128, KT * NT], F32, tag="cpart")
            call_d = dpool.tile([128, KT * NT], F32, tag="call", addr_space="Shared")

            gT = big.tile([128, KT, NCH * 512], gdt, tag="gT")
            onesf8 = big.tile([128, 2, 16], gdt, tag="onesf8")
            grow = big.tile([128, 8, D], BF16, tag="grow")
            rawall = big.tile([128, 8, D], BF16, tag="rawall")
            oh = big.tile([128, 8 * NT], BF16, tag="oh")
            wm = big.tile([128, 4 * 512], BF16, tag="wm")
            idn = big.tile([128, 128], F32, tag="idn")
            onesw = big.tile([128, 1], BF16, tag="onesw")
            e8 = big.tile([8, 8 * 128], BF16, tag="e8")
            e16 = big.tile([16, NCH * 128], BF16, tag="e16")
            sclT = big.tile([8, 128], BF16, tag="sclT")
            srow2 = big.tile([16, 512], BF16, tag="srow2")
            racc = big.tile([128, 80], F32, tag="racc")
            ssq = big.tile([128, 8], F32, tag="ssq")
            nrm = big.tile([128, 8], F32, tag="nrm")
            scl = big.tile([128, 8], F32, tag="scl")
            scl16 = big.tile([128, 8], F32, tag="scl16")
            stats = big.tile([128, 24], F32, tag="stats")
            Cst = big.tile([128, KT * NT], F32, tag="Cst")
            Cf8 = big.tile([128, KT * NT], gdt, tag="Cf8")
            warm = big.tile([128, 2], F32, tag="warm")

            # ---- raw rows first (they gate the whole scl chain), then consts ----
            for rt in range(8):
                nc.sync.dma_start(rawall[:, rt, :], feat[rt * 128 : (rt + 1) * 128, :])
            nc.sync.dma_start(idn[:, :], identr[:, :])
            nc.sync.dma_start(oh[:, :], ohr[:, :])
            nc.sync.dma_start(onesw[:, :], onesr[:, :])
            nc.sync.dma_start(e8[:, :], e8r[:, :])
            nc.sync.dma_start(e16[:, :], e16r[:, :])
            nc.vector.memset(warm[:, :], 1.0)
            nc.vector.memset(onesf8[:, :, :], 1.0)
            nc.vector.memset(racc[:, :], 0.0)
            nc.scalar.activation(warm[:, 0:1], warm[:, 1:2], AF.Sqrt)
            for q in range(4):
                wmq = wm[:, q * 512 : (q + 1) * 512]
                nc.gpsimd.memset(wmq, float(NEG))
                nc.gpsimd.affine_select(
                    out=wmq,
                    in_=wmq,
                    compare_op=mybir.AluOpType.is_equal,
                    fill=0.0,
                    base=-128 * q,
                    channel_multiplier=-1,
                    pattern=[[1, 512]],
                )

            # ---- ssq; scl ----
            for rt in range(8):
                dump = scr.tile([128, D], BF16, tag="dump")
                nc.scalar.activation(
                    dump[:, :],
                    rawall[:, rt, :],
                    AF.Square,
                    accum_out=ssq[:, rt : rt + 1],
                )
            nc.scalar.activation(nrm[:, :], ssq[:, :], AF.Sqrt, scale=float(T))
            nc.vector.tensor_scalar_max(nrm[:, :], nrm[:, :], float(np.sqrt(T) * 1e-12))
            nc.vector.reciprocal(scl[:, :], nrm[:, :])
            nc.vector.tensor_scalar_mul(scl16[:, :], scl[:, :], float(S))
            nc.scalar.activation(warm[:, 0:1], warm[:, 1:2], AF.Exp)

            # ---- grow (bf16 normalized own rows, for C) ----
            for rt in range(8):
                nc.vector.tensor_scalar_mul(
                    grow[:, rt, :], rawall[:, rt, :], scl[:, rt : rt + 1]
                )

            # ---- scl16 -> sclT [8,128]; AllGather ----
            tp = paux.tile([128, 512], F32, tag="aux")
            nc.tensor.transpose(tp[0:8, 0:128], scl16[:, :], idn[:, :])
            nc.vector.tensor_copy(sclT[:, :], tp[0:8, 0:128])
            nc.sync.dma_start(scl_d[:, :], sclT[:, :])
            nc.gpsimd.collective_compute(
                "AllGather",
                mybir.AluOpType.bypass,
                replica_groups=[list(range(NC))],
                ins=[scl_d.opt()],
                outs=[scl_all.opt()],
            )

            # ---- C_part on PE (overlaps barrier/AllGather); AllReduce ----
            for dt in range(KT):
                cps = paux.tile([128, 512], F32, tag="aux")
                for jt in range(8):
                    nc.tensor.matmul(
                        cps[:, 0:NT],
                        grow[:, jt, dt * 128 : (dt + 1) * 128],
                        oh[:, jt * NT : (jt + 1) * NT],
                        start=(jt == 0),
                        stop=(jt == 7),
                    )
                nc.vector.tensor_copy(Cst[:, dt * NT : (dt + 1) * NT], cps[:, 0:NT])
            nc.sync.dma_start(cpart_d[:, :], Cst[:, :])
            nc.gpsimd.collective_compute(
                "AllReduce",
                ADD,
                replica_groups=[list(range(NC))],
                ins=[cpart_d.opt()],
                outs=[call_d.opt()],
            )
            # ---- helpers ----
            def emit_normalize(l, sb):
                stg = stage.tile([128, KT, 512], BF16, tag="stg")
                for kt in range(KT):
                    nc.sync.dma_start(
                        stg[:, kt, :],
                        ftloc[kt * 128 : (kt + 1) * 128, l * 512 : (l + 1) * 512],
                    )
                for kt in range(KT):
                    nc.vector.tensor_mul(
                        gT[:, kt, l * 512 : (l + 1) * 512], stg[:, kt, :], sb[:, :]
                    )

            def emit_run(i_loc, d):
                l = i_loc + d
                o = l * 512
                colacc = None
                if d >= 1:
                    colacc = pcol.tile([1, 512], F32, tag="colacc")
                eb = None
                for q in range(4):
                    mt = 4 * i_loc + q
                    m = mt * 128
                    sp = psim.tile([128, 512], F32, tag="sim")
                    if USE_FP8:
                        for kp in range(4):
                            nc.tensor.matmul(
                                sp[:, :],
                                gT[:, 2 * kp : 2 * kp + 2, m : m + 128],
                                gT[:, 2 * kp : 2 * kp + 2, o : o + 512],
                                start=(kp == 0),
                                stop=(kp == 3),
                                perf_mode=DR,
                            )
                    else:
                        for kt in range(KT):
                            nc.tensor.matmul(
                                sp[:, :],
                                gT[:, kt, m : m + 128],
                                gT[:, kt, o : o + 512],
                                start=(kt == 0),
                                stop=(kt == KT - 1),
                            )
                    if d == 0:
                        nc.vector.tensor_add(
                            sp[:, :], sp[:, :], wm[:, q * 512 : (q + 1) * 512]
                        )
                    if d >= 1 and USE_FP8:
                        # exp tiles land in fp8 pairs; one DoubleRow ones-MM
                        # sums two tiles' columns at once
                        if q % 2 == 0:
                            eb = epool.tile([128, 2, 512], gdt, tag="eb")
                        nc.scalar.activation(
                            eb[:, q % 2, :],
                            sp[:, :],
                            AF.Exp,
                            scale=float(1.0 / (S * S)),
                            accum_out=racc[:, mt * 10 + d : mt * 10 + d + 1],
                        )
                        if q % 2 == 1:
                            nc.tensor.matmul(
                                colacc[:, :],
                                onesf8[:, :, 0:1],
                                eb[:, :, :],
                                start=(q == 1),
                                stop=(q == 3),
                                perf_mode=DR,
                                skip_group_check=True,
                            )
                    else:
                        ebs = epool.tile([128, 2, 512], gdt, tag="eb")
                        nc.scalar.activation(
                            ebs[:, 0, :],
                            sp[:, :],
                            AF.Exp,
                            scale=float(1.0 / (S * S)),
                            accum_out=racc[:, mt * 10 + d : mt * 10 + d + 1],
                        )
                        if d >= 1:
                            nc.tensor.matmul(
                                colacc[:, :],
                                onesw[:, :],
                                ebs[:, 0, :],
                                start=(q == 0),
                                stop=(q == 3),
                                skip_group_check=True,
                            )
                if d >= 1:
                    # drain colsum; ACT early (DVE busy normalizing), DVE late
                    csb = cpool.tile([1, 512], F32, tag="csb")
                    if i_loc + d <= 5:
                        nc.scalar.copy(csb[:, :], colacc[:, :])
                    else:
                        nc.vector.tensor_copy(csb[:, :], colacc[:, :])
                    ridx = i_loc * 8 + (d - 1)
                    nc.sync.dma_start(den_c_o[ridx : ridx + 1, :], csb[:, :])

            # ---- own chunks 0,1: selector from local sclT; normalize ----
            for l in range(2):
                sb = spool.tile([128, 512], BF16, tag="S")
                ax = paux.tile([128, 512], F32, tag="aux")
                for j in range(4):
                    q = 4 * l + j
                    nc.tensor.matmul(
                        ax[:, j * 128 : (j + 1) * 128],
                        e8[:, q * 128 : (q + 1) * 128],
                        sclT[:, :],
                        start=True,
                        stop=True,
                    )
                nc.vector.tensor_copy(sb[:, :], ax[:, :])
                emit_normalize(l, sb)

            # ---- runs on own chunks (no AllGather dependency) ----
            for (i_loc, d) in RUNS:
                if i_loc + d < 2:
                    emit_run(i_loc, d)

            # ---- gathered scales: selectors + normalize for chunks 2..9 ----
            nc.sync.dma_start(srow2[:, :], scl_all[:, :])
            for l in range(2, NCH):
                sb = spool.tile([128, 512], BF16, tag="S")
                ax = paux.tile([128, 512], F32, tag="aux")
                nc.tensor.matmul(
                    ax[:, :],
                    e16[:, l * 128 : (l + 1) * 128],
                    srow2[:, :],
                    start=True,
                    stop=True,
                )
                nc.vector.tensor_copy(sb[:, :], ax[:, :])
                emit_normalize(l, sb)

            # ---- remaining runs (positives slotted mid-stream) ----
            rem = [r for r in RUNS if r[0] + r[1] >= 2]
            for (i_loc, d) in rem[:8]:
                emit_run(i_loc, d)

            # ---- positives (C AllReduce long done by now) ----
            nc.sync.dma_start(Cst[:, :], call_d[:, :])
            nc.vector.tensor_copy(Cf8[:, :], Cst[:, :])
            for mt in range(8):
                pp = paux.tile([128, 512], F32, tag="aux")
                for kt in range(KT):
                    nc.tensor.matmul(
                        pp[:, 0:NT],
                        gT[:, kt, mt * 128 : (mt + 1) * 128],
                        Cf8[:, kt * NT : (kt + 1) * NT],
                        start=(kt == 0),
                        stop=(kt == KT - 1),
                    )
                scr2 = scr.tile([128, NT], F32, tag="pscr")
                nc.vector.tensor_mul(
                    scr2[:, :], pp[:, 0:NT], oh[:, mt * NT : (mt + 1) * NT]
                )
                nc.vector.reduce_sum(stats[:, 16 + mt : 17 + mt], scr2[:, :], axis=AX)

            for (i_loc, d) in rem[8:]:
                emit_run(i_loc, d)

            # ---- epilogue: den reductions; outputs ----
            for mt in range(8):
                nc.vector.reduce_sum(
                    stats[:, mt : mt + 1], racc[:, mt * 10 : mt * 10 + 8], axis=AX
                )
                nc.vector.tensor_copy(
                    stats[:, 8 + mt : 9 + mt], racc[:, mt * 10 + 8 : mt * 10 + 9]
                )
            nc.sync.dma_start(stats_o[:, :], stats[:, :])

    nc.compile()
    return nc


_NC_CACHE = None


def _get_program():
    global _NC_CACHE
    if _NC_CACHE is None:
        _NC_CACHE = build_program()
    return _NC_CACHE


def _build_inmaps(f, t):
    f_bf = f.astype(BF16_NP)
    OH = (t[:, None] == np.arange(NT)[None, :]).astype(BF16_NP)
    identity = np.eye(128, dtype=np.float32)
    ones128 = np.ones((128, 1), BF16_NP)
    e8 = np.zeros((8, 8 * 128), BF16_NP)
    for q in range(8):
        e8[q, q * 128 : (q + 1) * 128] = 1
    in_maps = []
    for c in range(NC):
        rot = (np.arange(NCH * 512) + 1024 * c) % N
        ftl = np.ascontiguousarray(f_bf[rot].T)
        e16 = np.zeros((16, NCH * 128), BF16_NP)
        for l in range(NCH):
            g = (l + 2 * c) % 16
            e16[g, l * 128 : (l + 1) * 128] = 1
        rows = slice(c * 1024, (c + 1) * 1024)
        oh_pm = np.ascontiguousarray(
            OH[rows].reshape(8, 128, NT).transpose(1, 0, 2).reshape(128, 8 * NT)
        )
        in_maps.append(
            {
                "ftloc": ftl,
                "feat_rows": np.ascontiguousarray(f_bf[rows]),
                "oh_rows": oh_pm,
                "ident": identity,
                "ones128": ones128,
                "e8sel": e8,
                "e16sel": e16,
            }
        )
    return in_maps


def _combine(res, t):
    S = S_FP8 if USE_FP8 else 1.0
    den = np.zeros(N, np.float64)
    pos = np.zeros(N, np.float64)
    for c in range(NC):
        st = np.asarray(res[c]["stats_o"], np.float64)
        dm, da, po = st[:, 0:8], st[:, 8:16], st[:, 16:24]
        dc = np.asarray(res[c]["den_col_o"], np.float64)
        base = 1024 * c
        den[base : base + 1024] += (dm + 0.5 * da).T.ravel()
        pos[base : base + 1024] = po.T.ravel() / S
        for i_loc in range(2):
            for d in range(1, 9):
                b = (2 * c + i_loc + d) % 16
                w = 0.5 if d == 8 else 1.0
                den[512 * b : 512 * b + 512] += w * dc[i_loc * 8 + (d - 1)]
    hist = np.bincount(t, minlength=NT)
    cnt = hist[t] - 1
    valid = cnt > 0
    inv = 1.0 / np.maximum(cnt, 1)
    pm = (pos - 1.0 / T) * inv
    loss = -np.log(np.exp(pm) / den + EPS)
    vc = int(valid.sum())
    return np.float32((loss * valid).sum() / vc) if vc > 0 else np.float32(0.0)


def kernel(features, element_types):
    f = np.ascontiguousarray(np.asarray(features), dtype=np.float32)
    t = np.asarray(element_types).astype(np.int64)
    assert f.shape == (N, D) and t.shape == (N,)
    in_maps = _build_inmaps(f, t)
    nc = _get_program()
    res = run_bass_kernel_spmd(nc, in_maps, list(range(NC))).results
    return _combine(res, t)


# revision 4
# speedup vs baseline: 1.0323x; 1.0323x over previous
"""Supervised contrastive loss kernel v3 — Trainium2, 8 cores, Bass/Tile.

Same SPMD-uniform cyclic-symmetric scheme as v2 (see kernel_v2.py docstring)
plus:
  - feat_rows shipped bf16 (faster start, 2x DVE ssq rate)
  - diagonal mask generated on device via affine_select (no wmask input)
  - class-sum matrix C cast to fp8 so the positives matmul reads gT directly
    (drops the gTo bf16 copy and its 16 DVE mults)
  - normalize multiplies alternate DVE/GpSimd (two engines share the work)
  - bulk DMAs issued from the SP (sync) HWDGE queue instead of gpsimd
  - positives emitted before the main loop so they fill the post-AllReduce
    PE gap
NOTE: tensor_tensor_reduce + collective_compute in one program deadlocks on
HW (bisected 2026-08-08) — stick to tensor_mul + reduce_sum.
"""

import numpy as np
import ml_dtypes

import concourse.bass as bass
import concourse.bacc as bacc
import concourse.mybir as mybir
from concourse import tile
from concourse.bass_utils import run_bass_kernel_spmd

N, D, NT, NC = 8192, 1024, 32, 8
KT = D // 128
NCH = 10
T = 0.07
EPS = 1e-10
NEG = -1.0e12

USE_FP8 = True
S_FP8 = 16.0

F32 = mybir.dt.float32
BF16 = mybir.dt.bfloat16
FP8 = mybir.dt.float8e4
BF16_NP = ml_dtypes.bfloat16

RUNS = sorted(
    [(i, d) for i in range(2) for d in range(9)], key=lambda r: (r[0] + r[1], r[0])
)


def build_program():
    S = S_FP8 if USE_FP8 else 1.0
    gdt = FP8 if USE_FP8 else BF16

    nc = bacc.Bacc(None, target_bir_lowering=False, debug=False)
    ftloc = nc.dram_tensor("ftloc", [D, NCH * 512], FP8, kind="ExternalInput")
    feat = nc.dram_tensor("feat_rows", [1024, D], BF16, kind="ExternalInput")
    ohr = nc.dram_tensor("oh_rows", [128, 8 * NT], BF16, kind="ExternalInput")
    identr = nc.dram_tensor("ident", [128, 128], F32, kind="ExternalInput")
    onesr = nc.dram_tensor("ones128", [128, 1], BF16, kind="ExternalInput")
    e8r = nc.dram_tensor("e8sel", [8, 8 * 128], BF16, kind="ExternalInput")
    e16r = nc.dram_tensor("e16sel", [16, NCH * 128], BF16, kind="ExternalInput")

    stats_o = nc.dram_tensor("stats_o", [128, 24], F32, kind="ExternalOutput")
    den_c_o = nc.dram_tensor("den_col_o", [16, 512], F32, kind="ExternalOutput")

    AX = mybir.AxisListType.X
    ADD = mybir.AluOpType.add
    AF = mybir.ActivationFunctionType
    DR = mybir.MatmulPerfMode.DoubleRow

    with tile.TileContext(nc) as tc:
        with (
            tc.tile_pool(name="dram", bufs=1, space="DRAM") as dpool,
            tc.tile_pool(name="big", bufs=1) as big,
            tc.tile_pool(name="stage", bufs=3) as stage,
            tc.tile_pool(name="spool", bufs=2) as spool,
            tc.tile_pool(name="epool", bufs=3) as epool,
            tc.tile_pool(name="cpool", bufs=2) as cpool,
            tc.tile_pool(name="scr", bufs=2) as scr,
            tc.tile_pool(name="psim", bufs=5, space="PSUM") as psim,
            tc.tile_pool(name="pcol", bufs=2, space="PSUM") as pcol,
            tc.tile_pool(name="paux", bufs=1, space="PSUM") as paux,
        ):
            scl_d = dpool.tile([8, 128], BF16, tag="scl_d")
            scl_all = dpool.tile([16, 512], BF16, tag="scl_all", addr_space="Shared")
            cpart_d = dpool.tile([128, KT * NT], F32, tag="cpart")
            call_d = dpool.tile([128, KT * NT], F32, tag="call", addr_space="Shared")

            gT = big.tile([128, KT, NCH * 512], gdt, tag="gT")
            onesf8 = big.tile([128, 2, 16], gdt, tag="onesf8")
            grow = big.tile([128, 8, D], BF16, tag="grow")
            rawall = big.tile([128, 8, D], BF16, tag="rawall")
            oh = big.tile([128, 8 * NT], BF16, tag="oh")
            wm = big.tile([128, 4 * 512], BF16, tag="wm")
            idn = big.tile([128, 128], F32, tag="idn")
            onesw = big.tile([128, 1], BF16, tag="onesw")
            e8 = big.tile([8, 8 * 128], BF16, tag="e8")
            e16 = big.tile([16, NCH * 128], BF16, tag="e16")
            sclT = big.tile([8, 128], BF16, tag="sclT")
            srow2 = big.tile([16, 512], BF16, tag="srow2")
            racc = big.tile([128, 80], F32, tag="racc")
            ssq = big.tile([128, 8], F32, tag="ssq")
            nrm = big.tile([128, 8], F32, tag="nrm")
            scl = big.tile([128, 8], F32, tag="scl")
            scl16 = big.tile([128, 8], F32, tag="scl16")
            stats = big.tile([128, 24], F32, tag="stats")
            Cst = big.tile([128, KT * NT], F32, tag="Cst")
            Cf8 = big.tile([128, KT * NT], gdt, tag="Cf8")
            warm = big.tile([128, 2], F32, tag="warm")

            # ---- raw rows first (they gate the whole scl chain), then consts ----
            for rt in range(8):
                nc.sync.dma_start(rawall[:, rt, :], feat[rt * 128 : (rt + 1) * 128, :])
            nc.sync.dma_start(idn[:, :], identr[:, :])
            nc.sync.dma_start(oh[:, :], ohr[:, :])
            nc.sync.dma_start(onesw[:, :], onesr[:, :])
            nc.sync.dma_start(e8[:, :], e8r[:, :])
            nc.sync.dma_start(e16[:, :], e16r[:, :])
            nc.vector.memset(warm[:, :], 1.0)
            nc.vector.memset(onesf8[:, :, :], 1.0)
            nc.vector.memset(racc[:, :], 0.0)
            nc.scalar.activation(warm[:, 0:1], warm[:, 1:2], AF.Sqrt)
            for q in range(4):
                wmq = wm[:, q * 512 : (q + 1) * 512]
                nc.gpsimd.memset(wmq, float(NEG))
                nc.gpsimd.affine_select(
                    out=wmq,
                    in_=wmq,
                    compare_op=mybir.AluOpType.is_equal,
                    fill=0.0,
                    base=-128 * q,
                    channel_multiplier=-1,
                    pattern=[[1, 512]],
                )

            # ---- ssq; scl ----
            for rt in range(8):
                dump = scr.tile([128, D], BF16, tag="dump")
                nc.scalar.activation(
                    dump[:, :],
                    rawall[:, rt, :],
                    AF.Square,
                    accum_out=ssq[:, rt : rt + 1],
                )
            nc.scalar.activation(nrm[:, :], ssq[:, :], AF.Sqrt, scale=float(T))
            nc.vector.tensor_scalar_max(nrm[:, :], nrm[:, :], float(np.sqrt(T) * 1e-12))
            nc.vector.reciprocal(scl[:, :], nrm[:, :])
            nc.vector.tensor_scalar_mul(scl16[:, :], scl[:, :], float(S))
            nc.scalar.activation(warm[:, 0:1], warm[:, 1:2], AF.Exp)

            # ---- grow (bf16 normalized own rows, for C) ----
            for rt in range(8):
                nc.vector.tensor_scalar_mul(
                    grow[:, rt, :], rawall[:, rt, :], scl[:, rt : rt + 1]
                )

            # ---- scl16 -> sclT [8,128]; AllGather ----
            tp = paux.tile([128, 512], F32, tag="aux")
            nc.tensor.transpose(tp[0:8, 0:128], scl16[:, :], idn[:, :])
            nc.vector.tensor_copy(sclT[:, :], tp[0:8, 0:128])
            nc.sync.dma_start(scl_d[:, :], sclT[:, :])
            nc.gpsimd.collective_compute(
                "AllGather",
                mybir.AluOpType.bypass,
                replica_groups=[list(range(NC))],
                ins=[scl_d.opt()],
                outs=[scl_all.opt()],
            )

            # ---- C_part on PE (overlaps barrier/AllGather); AllReduce ----
            for dt in range(KT):
                cps = paux.tile([128, 512], F32, tag="aux")
                for jt in range(8):
                    nc.tensor.matmul(
                        cps[:, 0:NT],
                        grow[:, jt, dt * 128 : (dt + 1) * 128],
                        oh[:, jt * NT : (jt + 1) * NT],
                        start=(jt == 0),
                        stop=(jt == 7),
                    )
                nc.vector.tensor_copy(Cst[:, dt * NT : (dt + 1) * NT], cps[:, 0:NT])
            nc.sync.dma_start(cpart_d[:, :], Cst[:, :])
            nc.gpsimd.collective_compute(
                "AllReduce",
                ADD,
                replica_groups=[list(range(NC))],
                ins=[cpart_d.opt()],
                outs=[call_d.opt()],
            )
            # ---- helpers ----
            def emit_normalize(l, sb):
                stg = stage.tile([128, KT, 512], FP8, tag="stg")
                for kt in range(KT):
                    nc.sync.dma_start(
                        stg[:, kt, :],
                        ftloc[kt * 128 : (kt + 1) * 128, l * 512 : (l + 1) * 512],
                    )
                for kt in range(KT):
                    nc.vector.tensor_mul(
                        gT[:, kt, l * 512 : (l + 1) * 512], stg[:, kt, :], sb[:, :]
                    )

            def emit_run(i_loc, d):
                l = i_loc + d
                o = l * 512
                colacc = None
                if d >= 1:
                    colacc = pcol.tile([1, 512], F32, tag="colacc")
                eb = None
                for q in range(4):
                    mt = 4 * i_loc + q
                    m = mt * 128
                    sp = psim.tile([128, 512], F32, tag="sim")
                    if USE_FP8:
                        for kp in range(4):
                            nc.tensor.matmul(
                                sp[:, :],
                                gT[:, 2 * kp : 2 * kp + 2, m : m + 128],
                                gT[:, 2 * kp : 2 * kp + 2, o : o + 512],
                                start=(kp == 0),
                                stop=(kp == 3),
                                perf_mode=DR,
                            )
                    else:
                        for kt in range(KT):
                            nc.tensor.matmul(
                                sp[:, :],
                                gT[:, kt, m : m + 128],
                                gT[:, kt, o : o + 512],
                                start=(kt == 0),
                                stop=(kt == KT - 1),
                            )
                    if d == 0:
                        nc.vector.tensor_add(
                            sp[:, :], sp[:, :], wm[:, q * 512 : (q + 1) * 512]
                        )
                    if d >= 1 and USE_FP8:
                        # exp tiles land in fp8 pairs; one DoubleRow ones-MM
                        # sums two tiles' columns at once
                        if q % 2 == 0:
                            eb = epool.tile([128, 2, 512], gdt, tag="eb")
                        nc.scalar.activation(
                            eb[:, q % 2, :],
                            sp[:, :],
                            AF.Exp,
                            scale=float(1.0 / (S * S)),
                            accum_out=racc[:, mt * 10 + d : mt * 10 + d + 1],
                        )
                        if q % 2 == 1:
                            nc.tensor.matmul(
                                colacc[:, :],
                                onesf8[:, :, 0:1],
                                eb[:, :, :],
                                start=(q == 1),
                                stop=(q == 3),
                                perf_mode=DR,
                                skip_group_check=True,
                            )
                    else:
                        ebs = epool.tile([128, 2, 512], gdt, tag="eb")
                        nc.scalar.activation(
                            ebs[:, 0, :],
                            sp[:, :],
                            AF.Exp,
                            scale=float(1.0 / (S * S)),
                            accum_out=racc[:, mt * 10 + d : mt * 10 + d + 1],
                        )
                        if d >= 1:
                            nc.tensor.matmul(
                                colacc[:, :],
                                onesw[:, :],
                                ebs[:, 0, :],
                                start=(q == 0),
                                stop=(q == 3),
                                skip_group_check=True,
                            )
                if d >= 1:
                    # drain colsum; ACT early (DVE busy normalizing), DVE late
                    csb = cpool.tile([1, 512], F32, tag="csb")
                    if i_loc + d <= 5:
                        nc.scalar.copy(csb[:, :], colacc[:, :])
                    else:
                        nc.vector.tensor_copy(csb[:, :], colacc[:, :])
                    ridx = i_loc * 8 + (d - 1)
                    nc.sync.dma_start(den_c_o[ridx : ridx + 1, :], csb[:, :])

            # ---- own chunks 0,1: selector from local sclT; normalize ----
            for l in range(2):
                sb = spool.tile([128, 512], BF16, tag="S")
                ax = paux.tile([128, 512], F32, tag="aux")
                for j in range(4):
                    q = 4 * l + j
                    nc.tensor.matmul(
                        ax[:, j * 128 : (j + 1) * 128],
                        e8[:, q * 128 : (q + 1) * 128],
                        sclT[:, :],
                        start=True,
                        stop=True,
                    )
                nc.vector.tensor_copy(sb[:, :], ax[:, :])
                emit_normalize(l, sb)

            # ---- runs on own chunks (no AllGather dependency) ----
            for (i_loc, d) in RUNS:
                if i_loc + d < 2:
                    emit_run(i_loc, d)

            # ---- gathered scales: selectors + normalize for chunks 2..9 ----
            nc.sync.dma_start(srow2[:, :], scl_all[:, :])
            for l in range(2, NCH):
                sb = spool.tile([128, 512], BF16, tag="S")
                ax = paux.tile([128, 512], F32, tag="aux")
                nc.tensor.matmul(
                    ax[:, :],
                    e16[:, l * 128 : (l + 1) * 128],
                    srow2[:, :],
                    start=True,
                    stop=True,
                )
                nc.vector.tensor_copy(sb[:, :], ax[:, :])
                emit_normalize(l, sb)

            # ---- remaining runs (positives slotted mid-stream) ----
            rem = [r for r in RUNS if r[0] + r[1] >= 2]
            for (i_loc, d) in rem[:8]:
                emit_run(i_loc, d)

            # ---- positives (C AllReduce long done by now) ----
            nc.sync.dma_start(Cst[:, :], call_d[:, :])
            nc.vector.tensor_copy(Cf8[:, :], Cst[:, :])
            for mt in range(8):
                pp = paux.tile([128, 512], F32, tag="aux")
                for kt in range(KT):
                    nc.tensor.matmul(
                        pp[:, 0:NT],
                        gT[:, kt, mt * 128 : (mt + 1) * 128],
                        Cf8[:, kt * NT : (kt + 1) * NT],
                        start=(kt == 0),
                        stop=(kt == KT - 1),
                    )
                scr2 = scr.tile([128, NT], F32, tag="pscr")
                nc.vector.tensor_mul(
                    scr2[:, :], pp[:, 0:NT], oh[:, mt * NT : (mt + 1) * NT]
                )
                nc.vector.reduce_sum(stats[:, 16 + mt : 17 + mt], scr2[:, :], axis=AX)

            for (i_loc, d) in rem[8:-1]:
                emit_run(i_loc, d)
            # i_loc=0 rows are complete before the final (1,8) run
            for mt in range(4):
                nc.vector.reduce_sum(
                    stats[:, mt : mt + 1], racc[:, mt * 10 : mt * 10 + 8], axis=AX
                )
                nc.vector.tensor_copy(
                    stats[:, 8 + mt : 9 + mt], racc[:, mt * 10 + 8 : mt * 10 + 9]
                )
            emit_run(*rem[-1])

            # ---- epilogue: den reductions; outputs ----
            for mt in range(4, 8):
                nc.vector.reduce_sum(
                    stats[:, mt : mt + 1], racc[:, mt * 10 : mt * 10 + 8], axis=AX
                )
                nc.vector.tensor_copy(
                    stats[:, 8 + mt : 9 + mt], racc[:, mt * 10 + 8 : mt * 10 + 9]
                )
            nc.sync.dma_start(stats_o[:, :], stats[:, :])

    nc.compile()
    return nc


_NC_CACHE = None


def _get_program():
    global _NC_CACHE
    if _NC_CACHE is None:
        _NC_CACHE = build_program()
    return _NC_CACHE


FP8_NP = ml_dtypes.float8_e4m3


def _build_inmaps(f, t):
    f_bf = f.astype(BF16_NP)
    OH = (t[:, None] == np.arange(NT)[None, :]).astype(BF16_NP)
    identity = np.eye(128, dtype=np.float32)
    ones128 = np.ones((128, 1), BF16_NP)
    e8 = np.zeros((8, 8 * 128), BF16_NP)
    for q in range(8):
        e8[q, q * 128 : (q + 1) * 128] = 1
    in_maps = []
    for c in range(NC):
        rot = (np.arange(NCH * 512) + 1024 * c) % N
        ftl = np.ascontiguousarray(f[rot].astype(FP8_NP).T)
        e16 = np.zeros((16, NCH * 128), BF16_NP)
        for l in range(NCH):
            g = (l + 2 * c) % 16
            e16[g, l * 128 : (l + 1) * 128] = 1
        rows = slice(c * 1024, (c + 1) * 1024)
        oh_pm = np.ascontiguousarray(
            OH[rows].reshape(8, 128, NT).transpose(1, 0, 2).reshape(128, 8 * NT)
        )
        in_maps.append(
            {
                "ftloc": ftl,
                "feat_rows": np.ascontiguousarray(f_bf[rows]),
                "oh_rows": oh_pm,
                "ident": identity,
                "ones128": ones128,
                "e8sel": e8,
                "e16sel": e16,
            }
        )
    return in_maps


def _combine(res, t):
    S = S_FP8 if USE_FP8 else 1.0
    den = np.zeros(N, np.float64)
    pos = np.zeros(N, np.float64)
    for c in range(NC):
        st = np.asarray(res[c]["stats_o"], np.float64)
        dm, da, po = st[:, 0:8], st[:, 8:16], st[:, 16:24]
        dc = np.asarray(res[c]["den_col_o"], np.float64)
        base = 1024 * c
        den[base : base + 1024] += (dm + 0.5 * da).T.ravel()
        pos[base : base + 1024] = po.T.ravel() / S
        for i_loc in range(2):
            for d in range(1, 9):
                b = (2 * c + i_loc + d) % 16
                w = 0.5 if d == 8 else 1.0
                den[512 * b : 512 * b + 512] += w * dc[i_loc * 8 + (d - 1)]
    hist = np.bincount(t, minlength=NT)
    cnt = hist[t] - 1
    valid = cnt > 0
    inv = 1.0 / np.maximum(cnt, 1)
    pm = (pos - 1.0 / T) * inv
    loss = -np.log(np.exp(pm) / den + EPS)
    vc = int(valid.sum())
    return np.float32((loss * valid).sum() / vc) if vc > 0 else np.float32(0.0)


def kernel(features, element_types):
    f = np.ascontiguousarray(np.asarray(features), dtype=np.float32)
    t = np.asarray(element_types).astype(np.int64)
    assert f.shape == (N, D) and t.shape == (N,)
    in_maps = _build_inmaps(f, t)
    nc = _get_program()
    res = run_bass_kernel_spmd(nc, in_maps, list(range(NC))).results
    return _combine(res, t)


# revision 5
# speedup vs baseline: 1.1555x; 1.1194x over previous
"""Supervised contrastive loss kernel v3 — Trainium2, 8 cores, Bass/Tile.

Same SPMD-uniform cyclic-symmetric scheme as v2 (see kernel_v2.py docstring)
plus:
  - feat_rows shipped bf16 (faster start, 2x DVE ssq rate)
  - diagonal mask generated on device via affine_select (no wmask input)
  - class-sum matrix C cast to fp8 so the positives matmul reads gT directly
    (drops the gTo bf16 copy and its 16 DVE mults)
  - normalize multiplies alternate DVE/GpSimd (two engines share the work)
  - bulk DMAs issued from the SP (sync) HWDGE queue instead of gpsimd
  - positives emitted before the main loop so they fill the post-AllReduce
    PE gap
NOTE: tensor_tensor_reduce + collective_compute in one program deadlocks on
HW (bisected 2026-08-08) — stick to tensor_mul + reduce_sum.
"""

import numpy as np
import ml_dtypes

import concourse.bass as bass
import concourse.bacc as bacc
import concourse.mybir as mybir
from concourse import tile
from concourse.bass_utils import run_bass_kernel_spmd

N, D, NT, NC = 8192, 1024, 32, 8
KT = D // 128
NCH = 10
T = 0.07
EPS = 1e-10
NEG = -1.0e12

USE_FP8 = True
S_FP8 = 16.0

F32 = mybir.dt.float32
BF16 = mybir.dt.bfloat16
FP8 = mybir.dt.float8e4
BF16_NP = ml_dtypes.bfloat16

RUNS = sorted(
    [(i, d) for i in range(2) for d in range(9)], key=lambda r: (r[0] + r[1], r[0])
)


def build_program():
    S = S_FP8 if USE_FP8 else 1.0
    gdt = FP8 if USE_FP8 else BF16

    nc = bacc.Bacc(None, target_bir_lowering=False, debug=False)
    ftloc = nc.dram_tensor("ftloc", [D, NCH * 512], FP8, kind="ExternalInput")
    feat = nc.dram_tensor("feat_rows", [1024, D], BF16, kind="ExternalInput")
    ohr = nc.dram_tensor("oh_rows", [128, 8 * NT], BF16, kind="ExternalInput")
    identr = nc.dram_tensor("ident", [128, 128], F32, kind="ExternalInput")
    onesr = nc.dram_tensor("ones128", [128, 1], BF16, kind="ExternalInput")
    e8r = nc.dram_tensor("e8sel", [8, 8 * 128], BF16, kind="ExternalInput")
    e16r = nc.dram_tensor("e16sel", [16, NCH * 128], BF16, kind="ExternalInput")

    stats_o = nc.dram_tensor("stats_o", [128, 24], F32, kind="ExternalOutput")
    den_c_o = nc.dram_tensor("den_col_o", [16, 512], F32, kind="ExternalOutput")

    AX = mybir.AxisListType.X
    ADD = mybir.AluOpType.add
    AF = mybir.ActivationFunctionType
    DR = mybir.MatmulPerfMode.DoubleRow

    with tile.TileContext(nc) as tc:
        with (
            tc.tile_pool(name="dram", bufs=1, space="DRAM") as dpool,
            tc.tile_pool(name="big", bufs=1) as big,
            tc.tile_pool(name="stage", bufs=3) as stage,
            tc.tile_pool(name="spool", bufs=2) as spool,
            tc.tile_pool(name="epool", bufs=3) as epool,
            tc.tile_pool(name="cpool", bufs=2) as cpool,
            tc.tile_pool(name="scr", bufs=2) as scr,
            tc.tile_pool(name="psim", bufs=5, space="PSUM") as psim,
            tc.tile_pool(name="pcol", bufs=2, space="PSUM") as pcol,
            tc.tile_pool(name="paux", bufs=1, space="PSUM") as paux,
        ):
            scl_d = dpool.tile([8, 128], BF16, tag="scl_d")
            scl_all = dpool.tile([16, 512], BF16, tag="scl_all", addr_space="Shared")
            cpart_d = dpool.tile([128, KT * NT], F32, tag="cpart")
            call_d = dpool.tile([128, KT * NT], F32, tag="call", addr_space="Shared")

            gT = big.tile([128, KT, NCH * 512], gdt, tag="gT")
            onesf8 = big.tile([128, 2, 16], gdt, tag="onesf8")
            grow = big.tile([128, 8, D], BF16, tag="grow")
            rawall = big.tile([128, 8, D], BF16, tag="rawall")
            oh = big.tile([128, 8 * NT], BF16, tag="oh")
            wm = big.tile([128, 4 * 512], BF16, tag="wm")
            idn = big.tile([128, 128], F32, tag="idn")
            onesw = big.tile([128, 1], BF16, tag="onesw")
            e8 = big.tile([8, 8 * 128], BF16, tag="e8")
            e16 = big.tile([16, NCH * 128], BF16, tag="e16")
            sclT = big.tile([8, 128], BF16, tag="sclT")
            srow2 = big.tile([16, 512], BF16, tag="srow2")
            racc = big.tile([128, 80], F32, tag="racc")
            ssq = big.tile([128, 8], F32, tag="ssq")
            nrm = big.tile([128, 8], F32, tag="nrm")
            scl = big.tile([128, 8], F32, tag="scl")
            scl16 = big.tile([128, 8], F32, tag="scl16")
            stats = big.tile([128, 24], F32, tag="stats")
            Cst = big.tile([128, KT * NT], F32, tag="Cst")
            Cf8 = big.tile([128, KT * NT], gdt, tag="Cf8")
            warm = big.tile([128, 2], F32, tag="warm")

            # ---- raw rows first (they gate the whole scl chain), then consts ----
            for rt in range(8):
                nc.sync.dma_start(rawall[:, rt, :], feat[rt * 128 : (rt + 1) * 128, :])
            nc.sync.dma_start(idn[:, :], identr[:, :])
            nc.sync.dma_start(oh[:, :], ohr[:, :])
            nc.sync.dma_start(onesw[:, :], onesr[:, :])
            nc.sync.dma_start(e8[:, :], e8r[:, :])
            nc.sync.dma_start(e16[:, :], e16r[:, :])
            nc.vector.memset(warm[:, :], 1.0)
            nc.vector.memset(onesf8[:, :, :], 1.0)
            nc.vector.memset(racc[:, :], 0.0)
            nc.scalar.activation(warm[:, 0:1], warm[:, 1:2], AF.Sqrt)
            for q in range(4):
                wmq = wm[:, q * 512 : (q + 1) * 512]
                nc.gpsimd.memset(wmq, float(NEG))
                nc.gpsimd.affine_select(
                    out=wmq,
                    in_=wmq,
                    compare_op=mybir.AluOpType.is_equal,
                    fill=0.0,
                    base=-128 * q,
                    channel_multiplier=-1,
                    pattern=[[1, 512]],
                )

            # ---- ssq; scl ----
            for rt in range(8):
                dump = scr.tile([128, D], BF16, tag="dump")
                nc.scalar.activation(
                    dump[:, :],
                    rawall[:, rt, :],
                    AF.Square,
                    accum_out=ssq[:, rt : rt + 1],
                )
            nc.scalar.activation(nrm[:, :], ssq[:, :], AF.Sqrt, scale=float(T))
            nc.vector.tensor_scalar_max(nrm[:, :], nrm[:, :], float(np.sqrt(T) * 1e-12))
            nc.vector.reciprocal(scl[:, :], nrm[:, :])
            nc.vector.tensor_scalar_mul(scl16[:, :], scl[:, :], float(S))
            nc.scalar.activation(warm[:, 0:1], warm[:, 1:2], AF.Exp)

            # ---- grow (bf16 normalized own rows, for C) ----
            for rt in range(8):
                nc.vector.tensor_scalar_mul(
                    grow[:, rt, :], rawall[:, rt, :], scl[:, rt : rt + 1]
                )

            # ---- scl16 -> sclT [8,128]; AllGather ----
            tp = paux.tile([128, 512], F32, tag="aux")
            nc.tensor.transpose(tp[0:8, 0:128], scl16[:, :], idn[:, :])
            nc.vector.tensor_copy(sclT[:, :], tp[0:8, 0:128])
            nc.sync.dma_start(scl_d[:, :], sclT[:, :])
            nc.gpsimd.collective_compute(
                "AllGather",
                mybir.AluOpType.bypass,
                replica_groups=[list(range(NC))],
                ins=[scl_d.opt()],
                outs=[scl_all.opt()],
            )

            # ---- C_part on PE (overlaps barrier/AllGather); AllReduce ----
            for dt in range(KT):
                cps = paux.tile([128, 512], F32, tag="aux")
                for jt in range(8):
                    nc.tensor.matmul(
                        cps[:, 0:NT],
                        grow[:, jt, dt * 128 : (dt + 1) * 128],
                        oh[:, jt * NT : (jt + 1) * NT],
                        start=(jt == 0),
                        stop=(jt == 7),
                    )
                nc.vector.tensor_copy(Cst[:, dt * NT : (dt + 1) * NT], cps[:, 0:NT])
            nc.gpsimd.dma_start(cpart_d[:, :], Cst[:, :])
            nc.gpsimd.collective_compute(
                "AllReduce",
                ADD,
                replica_groups=[list(range(NC))],
                ins=[cpart_d.opt()],
                outs=[call_d.opt()],
            )
            # ---- helpers ----
            pending = []

            def flush_pending():
                while pending:
                    pending.pop(0)()

            def emit_normalize(l, sb):
                stg = stage.tile([128, KT, 512], FP8, tag="stg")
                for kt in range(KT):
                    nc.sync.dma_start(
                        stg[:, kt, :],
                        ftloc[kt * 128 : (kt + 1) * 128, l * 512 : (l + 1) * 512],
                    )
                for kt in range(KT):
                    nc.vector.tensor_mul(
                        gT[:, kt, l * 512 : (l + 1) * 512], stg[:, kt, :], sb[:, :]
                    )

            def emit_run(i_loc, d):
                l = i_loc + d
                o = l * 512
                colacc = None
                if d >= 1:
                    colacc = pcol.tile([1, 512], F32, tag="colacc")
                eb = None
                for q in range(4):
                    mt = 4 * i_loc + q
                    m = mt * 128
                    sp = psim.tile([128, 512], F32, tag="sim")
                    emitted_mm = True
                    if USE_FP8:
                        for kp in range(4):
                            nc.tensor.matmul(
                                sp[:, :],
                                gT[:, 2 * kp : 2 * kp + 2, m : m + 128],
                                gT[:, 2 * kp : 2 * kp + 2, o : o + 512],
                                start=(kp == 0),
                                stop=(kp == 3),
                                perf_mode=DR,
                            )
                    else:
                        for kt in range(KT):
                            nc.tensor.matmul(
                                sp[:, :],
                                gT[:, kt, m : m + 128],
                                gT[:, kt, o : o + 512],
                                start=(kt == 0),
                                stop=(kt == KT - 1),
                            )
                    flush_pending()
                    if d == 0:
                        nc.vector.tensor_add(
                            sp[:, :], sp[:, :], wm[:, q * 512 : (q + 1) * 512]
                        )
                    if d >= 1 and USE_FP8:
                        # exp tiles land in fp8 pairs; one DoubleRow ones-MM
                        # sums two tiles' columns at once
                        if q % 2 == 0:
                            eb = epool.tile([128, 2, 512], gdt, tag="eb")
                        nc.scalar.activation(
                            eb[:, q % 2, :],
                            sp[:, :],
                            AF.Exp,
                            scale=float(1.0 / (S * S)),
                            accum_out=racc[:, mt * 10 + d : mt * 10 + d + 1],
                        )
                        if q % 2 == 1:
                            def _colsum(eb=eb, colacc=colacc, q=q, i_loc=i_loc, d=d, l=l):
                                nc.tensor.matmul(
                                    colacc[:, :],
                                    onesf8[:, :, 0:1],
                                    eb[:, :, :],
                                    start=(q == 1),
                                    stop=(q == 3),
                                    perf_mode=DR,
                                    skip_group_check=True,
                                )
                                if q == 3:
                                    csb = cpool.tile([1, 512], F32, tag="csb")
                                    if l <= 5:
                                        nc.scalar.copy(csb[:, :], colacc[:, :])
                                    else:
                                        nc.vector.tensor_copy(csb[:, :], colacc[:, :])
                                    ridx = i_loc * 8 + (d - 1)
                                    nc.sync.dma_start(
                                        den_c_o[ridx : ridx + 1, :], csb[:, :]
                                    )
                            pending.append(_colsum)
                    else:
                        ebs = epool.tile([128, 2, 512], gdt, tag="eb")
                        nc.scalar.activation(
                            ebs[:, 0, :],
                            sp[:, :],
                            AF.Exp,
                            scale=float(1.0 / (S * S)),
                            accum_out=racc[:, mt * 10 + d : mt * 10 + d + 1],
                        )
                        if d >= 1:
                            nc.tensor.matmul(
                                colacc[:, :],
                                onesw[:, :],
                                ebs[:, 0, :],
                                start=(q == 0),
                                stop=(q == 3),
                                skip_group_check=True,
                            )


            # ---- own chunks 0,1: selector from local sclT; normalize ----
            for l in range(2):
                sb = spool.tile([128, 512], BF16, tag="S")
                ax = paux.tile([128, 512], F32, tag="aux")
                for j in range(4):
                    q = 4 * l + j
                    nc.tensor.matmul(
                        ax[:, j * 128 : (j + 1) * 128],
                        e8[:, q * 128 : (q + 1) * 128],
                        sclT[:, :],
                        start=True,
                        stop=True,
                    )
                nc.vector.tensor_copy(sb[:, :], ax[:, :])
                emit_normalize(l, sb)

            # ---- runs on own chunks (no AllGather dependency) ----
            for (i_loc, d) in RUNS:
                if i_loc + d < 2:
                    emit_run(i_loc, d)

            # ---- gathered scales: selectors + normalize for chunks 2..9 ----
            nc.sync.dma_start(srow2[:, :], scl_all[:, :])
            for l in range(2, NCH):
                sb = spool.tile([128, 512], BF16, tag="S")
                ax = paux.tile([128, 512], F32, tag="aux")
                nc.tensor.matmul(
                    ax[:, :],
                    e16[:, l * 128 : (l + 1) * 128],
                    srow2[:, :],
                    start=True,
                    stop=True,
                )
                nc.vector.tensor_copy(sb[:, :], ax[:, :])
                emit_normalize(l, sb)

            # ---- remaining runs (positives slotted mid-stream) ----
            rem = [r for r in RUNS if r[0] + r[1] >= 2]
            for (i_loc, d) in rem[:8]:
                emit_run(i_loc, d)

            # ---- positives (C AllReduce long done by now) ----
            nc.gpsimd.dma_start(Cst[:, :], call_d[:, :])
            nc.vector.tensor_copy(Cf8[:, :], Cst[:, :])
            for mt in range(8):
                pp = paux.tile([128, 512], F32, tag="aux")
                for kt in range(KT):
                    nc.tensor.matmul(
                        pp[:, 0:NT],
                        gT[:, kt, mt * 128 : (mt + 1) * 128],
                        Cf8[:, kt * NT : (kt + 1) * NT],
                        start=(kt == 0),
                        stop=(kt == KT - 1),
                    )
                scr2 = scr.tile([128, NT], F32, tag="pscr")
                nc.vector.tensor_mul(
                    scr2[:, :], pp[:, 0:NT], oh[:, mt * NT : (mt + 1) * NT]
                )
                nc.vector.reduce_sum(stats[:, 16 + mt : 17 + mt], scr2[:, :], axis=AX)

            for (i_loc, d) in rem[8:-1]:
                emit_run(i_loc, d)
            # i_loc=0 rows are complete before the final (1,8) run
            for mt in range(4):
                nc.vector.reduce_sum(
                    stats[:, mt : mt + 1], racc[:, mt * 10 : mt * 10 + 8], axis=AX
                )
                nc.vector.tensor_copy(
                    stats[:, 8 + mt : 9 + mt], racc[:, mt * 10 + 8 : mt * 10 + 9]
                )
            emit_run(*rem[-1])
            flush_pending()

            # ---- epilogue: den reductions; outputs ----
            for mt in range(4, 8):
                nc.vector.reduce_sum(
                    stats[:, mt : mt + 1], racc[:, mt * 10 : mt * 10 + 8], axis=AX
                )
                nc.vector.tensor_copy(
                    stats[:, 8 + mt : 9 + mt], racc[:, mt * 10 + 8 : mt * 10 + 9]
                )
            nc.sync.dma_start(stats_o[:, :], stats[:, :])

    nc.compile()
    return nc


_NC_CACHE = None


def _get_program():
    global _NC_CACHE
    if _NC_CACHE is None:
        _NC_CACHE = build_program()
    return _NC_CACHE


FP8_NP = ml_dtypes.float8_e4m3


def _build_inmaps(f, t):
    f_bf = f.astype(BF16_NP)
    OH = (t[:, None] == np.arange(NT)[None, :]).astype(BF16_NP)
    identity = np.eye(128, dtype=np.float32)
    ones128 = np.ones((128, 1), BF16_NP)
    e8 = np.zeros((8, 8 * 128), BF16_NP)
    for q in range(8):
        e8[q, q * 128 : (q + 1) * 128] = 1
    in_maps = []
    for c in range(NC):
        rot = (np.arange(NCH * 512) + 1024 * c) % N
        ftl = np.ascontiguousarray(f[rot].astype(FP8_NP).T)
        e16 = np.zeros((16, NCH * 128), BF16_NP)
        for l in range(NCH):
            g = (l + 2 * c) % 16
            e16[g, l * 128 : (l + 1) * 128] = 1
        rows = slice(c * 1024, (c + 1) * 1024)
        oh_pm = np.ascontiguousarray(
            OH[rows].reshape(8, 128, NT).transpose(1, 0, 2).reshape(128, 8 * NT)
        )
        in_maps.append(
            {
                "ftloc": ftl,
                "feat_rows": np.ascontiguousarray(f_bf[rows]),
                "oh_rows": oh_pm,
                "ident": identity,
                "ones128": ones128,
                "e8sel": e8,
                "e16sel": e16,
            }
        )
    return in_maps


def _combine(res, t):
    S = S_FP8 if USE_FP8 else 1.0
    den = np.zeros(N, np.float64)
    pos = np.zeros(N, np.float64)
    for c in range(NC):
        st = np.asarray(res[c]["stats_o"], np.float64)
        dm, da, po = st[:, 0:8], st[:, 8:16], st[:, 16:24]
        dc = np.asarray(res[c]["den_col_o"], np.float64)
        base = 1024 * c
        den[base : base + 1024] += (dm + 0.5 * da).T.ravel()
        pos[base : base + 1024] = po.T.ravel() / S
        for i_loc in range(2):
            for d in range(1, 9):
                b = (2 * c + i_loc + d) % 16
                w = 0.5 if d == 8 else 1.0
                den[512 * b : 512 * b + 512] += w * dc[i_loc * 8 + (d - 1)]
    hist = np.bincount(t, minlength=NT)
    cnt = hist[t] - 1
    valid = cnt > 0
    inv = 1.0 / np.maximum(cnt, 1)
    pm = (pos - 1.0 / T) * inv
    loss = -np.log(np.exp(pm) / den + EPS)
    vc = int(valid.sum())
    return np.float32((loss * valid).sum() / vc) if vc > 0 else np.float32(0.0)


def kernel(features, element_types):
    f = np.ascontiguousarray(np.asarray(features), dtype=np.float32)
    t = np.asarray(element_types).astype(np.int64)
    assert f.shape == (N, D) and t.shape == (N,)
    in_maps = _build_inmaps(f, t)
    nc = _get_program()
    res = run_bass_kernel_spmd(nc, in_maps, list(range(NC))).results
    return _combine(res, t)
